# revision 17
# baseline (speedup 1.0000x reference)
"""Trainium2 Bass kernel for the sparse-conv encoder (gnn_message_passing).

Design:
- 8-way SPMD over contiguous row slabs of each layer's output, with halo
  bands (each core redundantly computes the halo rows later layers gather),
  so there are NO collectives: every core runs an independent program on its
  own band of the point cloud.
- Layers 0+1 (conv0 then kmd0) are FUSED on the host: out0 is linear in
  host-known gathers of x, so out0 = relu(G1 @ Wbig + bd0) exactly, where
  G1[n] holds the 27x27 two-hop x-neighborhood values + per-tap hit bits
  (27x82 = 2214 features, shipped transposed in bf16) and
  Wbig[(k,(j,d)),:] = W0[j] @ Wd0[k] (hit rows b0 @ Wd0[k]) is composed in
  fp32. This removes the two worst gather layers entirely (the gather
  primitives cost ~45 ns per gathered row on the Q7 descriptor path,
  measured, so eliminating indices dominates all other optimizations).
- Layers 2-5 are gather-GEMMs: feature tables live in DRAM as bf16
  row-major with a zero row interleaved every 2049 stored rows. Per output
  chunk of 512 rows and per tap, a dma_gather(transpose=True) pulls the 512
  needed rows directly into SBUF **already transposed** ([C=128 part, 512]),
  which feeds nc.tensor.matmul as the stationary lhsT with the tap weight
  [C_in, C_out] streamed as rhs, accumulating the chunk's [512, C_out]
  output in PSUM across the 27 taps (bias seeded via a K=1 ones x bias
  matmul). dma_gather indices are int16, so each chunk reads through a
  32768-row window whose base advances affinely with the chunk index
  (base = chunk * BSTEP); the host shifts tables by PREPAD rows so one
  compile-time BSTEP per layer covers all 8 cores. Missing taps (-1) are
  remapped by the host to an in-window interleaved zero row.
"""
import hashlib
import os
import numpy as np

P = 128
CH = 512            # output rows per chunk
SEG = 2048          # real rows per zero-row segment (stored period SEG+1)
WIN = 32768         # int16 gather window (rows)
NTAP = 27
UNROLL = 4

_BUILD_CACHE = {}


# ---------------------------------------------------------------- NEFF cache
def _install_neff_cache():
    try:
        import concourse.bass_utils as bu
        import concourse.bass2jax as b2j
        if getattr(bu, "_ant_neff_cache_installed", False):
            return
        cache_dir = os.path.expanduser("~/.cache/ant_neff_cache")
        os.makedirs(cache_dir, exist_ok=True)
        orig = bu.compile_bir_kernel

        def cached(bir_json, tmpdir, neff_name="file.neff", **kw):
            if isinstance(bir_json, bytes):
                key = hashlib.sha256(bir_json).hexdigest()
            elif isinstance(bir_json, str):
                key = hashlib.sha256(bir_json.encode()).hexdigest()
            else:
                return orig(bir_json, tmpdir, neff_name=neff_name, **kw)
            path = os.path.join(cache_dir, key + ".neff")
            out_path = os.path.join(tmpdir, neff_name)
            if os.path.exists(path):
                import shutil
                shutil.copyfile(path, out_path)
                return out_path
            res = orig(bir_json, tmpdir, neff_name=neff_name, **kw)
            try:
                import shutil
                shutil.copyfile(res, path)
            except Exception:
                pass
            return res

        bu.compile_bir_kernel = cached
        b2j.compile_bir_kernel = cached
        bu._ant_neff_cache_installed = True
    except Exception:
        pass


# ---------------------------------------------------------------- host plan
def _np32(a):
    a = np.asarray(a)
    if a.dtype == np.int64:
        a = a.astype(np.int32)
    return a


def _hull(kmap, lo, hi):
    sub = kmap[:, lo:hi]
    v = sub[sub >= 0]
    if v.size == 0:
        return 0, 1
    return int(v.min()), int(v.max()) + 1


def _bands(kmaps, ncores=8):
    km0, kmd0, km1, kmd1, km2, kmd2 = kmaps

    def split(n):
        b = [round(c * n / ncores) for c in range(ncores + 1)]
        return [(b[c], b[c + 1]) for c in range(ncores)]

    owned0 = split(kmd0.shape[1])
    owned1 = split(kmd1.shape[1])
    owned2 = split(kmd2.shape[1])
    cores = []
    for c in range(ncores):
        o2 = owned2[c]
        b_h2 = _hull(kmd2, *o2)
        h_km2 = _hull(km2, *b_h2)
        o1 = owned1[c]
        b_out1 = (min(o1[0], h_km2[0]), max(o1[1], h_km2[1]))
        b_h1 = _hull(kmd1, *b_out1)
        h_km1 = _hull(km1, *b_h1)
        o0 = owned0[c]
        b_out0 = (min(o0[0], h_km1[0]), max(o0[1], h_km1[1]))
        b_h0 = _hull(kmd0, *b_out0)
        cores.append(dict(b_h0=b_h0, b_out0=b_out0, o0=o0,
                          b_h1=b_h1, b_out1=b_out1, o1=o1,
                          b_h2=b_h2, b_out2=o2, o2=o2))
    return cores


def _spos(r, prepad):
    """local real row -> stored row (zero row every SEG real rows)."""
    return prepad + 1 + r + r // SEG


def _solve_layer(kmap, out_bands, in_bands, writer_w_pad):
    """Pick BSTEP/PREPAD for one gather layer. Returns dict of constants."""
    ncores = len(out_bands)
    out_w = max(hi - lo for lo, hi in out_bands)
    nchunk = -(-out_w // CH)
    out_w_pad = nchunk * CH
    lo_l, hi_l, cks = [], [], []
    for c in range(ncores):
        olo, ohi = out_bands[c]
        ilo, _ = in_bands[c]
        w = ohi - olo
        sub = kmap[:, olo:ohi]
        loc = np.where(sub >= 0, (sub - ilo) + (sub - ilo) // SEG + 1, -1)
        for ck in range(nchunk):
            s, e = ck * CH, min((ck + 1) * CH, w)
            if s >= w:
                continue
            v = loc[:, s:e]
            v = v[v >= 0]
            if v.size == 0:
                continue
            lo_l.append(int(v.min()))
            hi_l.append(int(v.max()) + 1)
            cks.append(ck)
    lo_a = np.array(lo_l)
    hi_a = np.array(hi_l)
    ck_a = np.array(cks)

    def cost(b):
        m1 = max(0, int((ck_a * b - lo_a).max()))
        m2 = int((hi_a - ck_a * b).max())
        return m1 + m2, m1

    # convex in b -> ternary search over ints
    lo_b, hi_b = 0, 4096
    while hi_b - lo_b > 2:
        m1 = lo_b + (hi_b - lo_b) // 3
        m2 = hi_b - (hi_b - lo_b) // 3
        if cost(m1)[0] <= cost(m2)[0]:
            hi_b = m2
        else:
            lo_b = m1
    best = min(range(lo_b, hi_b + 1), key=lambda b: cost(b)[0])
    tot, prepad = cost(best)
    assert tot <= WIN - 64, f"window infeasible: {tot} > {WIN}"
    bstep = int(best)
    # stored table size: written rows + window reach
    stored_n = max(prepad + 1 + writer_w_pad + writer_w_pad // SEG + 1,
                   (nchunk - 1) * bstep + WIN, WIN)
    stored_n = -(-stored_n // P) * P
    return dict(nchunk=nchunk, out_w_pad=out_w_pad, bstep=bstep,
                prepad=prepad, stored_n=stored_n)


def _wrap_idx(rel):
    """rel [nchunk, NTAP, CH] int -> [nchunk*128, NTAP*CH//16] int16
    (16-partition wrap, replicated x8)."""
    nchunk = rel.shape[0]
    a = rel.reshape(nchunk, NTAP, CH // 16, 16).transpose(0, 3, 1, 2)
    a = a.reshape(nchunk, 1, 16, NTAP * (CH // 16))
    a = np.broadcast_to(a, (nchunk, 8, 16, NTAP * (CH // 16)))
    return np.ascontiguousarray(
        a.reshape(nchunk * P, NTAP * (CH // 16)), ).astype(np.int16)


def _layer_idx(kmap, out_band, in_band, cfg):
    """Build the wrapped int16 index buffer for one core of one layer."""
    olo, ohi = out_band
    ilo, _ = in_band
    w = ohi - olo
    nchunk, bstep, prepad = cfg["nchunk"], cfg["bstep"], cfg["prepad"]
    sub = kmap[:, olo:ohi].astype(np.int64)
    loc = np.where(sub >= 0, prepad + 1 + (sub - ilo) + (sub - ilo) // SEG, -1)
    full = np.full((NTAP, nchunk * CH), -1, np.int64)
    full[:, :w] = loc
    rel = np.empty((nchunk, NTAP, CH), np.int64)
    max_base = cfg["stored_n"] - WIN
    for ck in range(nchunk):
        base = ck * bstep
        assert 0 <= base <= max_base, (ck, base, max_base)
        g = max(0, -(-(base - prepad) // (SEG + 1)))
        zr = prepad + (SEG + 1) * g
        assert base <= zr < base + WIN and zr < cfg["stored_n"]
        v = full[:, ck * CH:(ck + 1) * CH] - base
        v = np.where(full[:, ck * CH:(ck + 1) * CH] < 0, zr - base, v)
        assert v.min() >= 0 and v.max() < WIN, (ck, v.min(), v.max())
        rel[ck] = v
    return _wrap_idx(rel)


def make_plan(inputs, ncores=8):
    import ml_dtypes
    bf16 = ml_dtypes.bfloat16
    x = np.asarray(inputs["x"], np.float32)
    kmaps = [_np32(inputs[k]) for k in
             ["km0", "kmd0", "km1", "kmd1", "km2", "kmd2"]]
    km0, kmd0, km1, kmd1, km2, kmd2 = kmaps
    cores = _bands(kmaps, ncores)

    # fused (layer0 + kmd0) output band: b_out0, padded
    out0_w = max(c["b_out0"][1] - c["b_out0"][0] for c in cores)
    nchunkf = -(-out0_w // CH)
    out0_w_pad = nchunkf * CH
    KB = (NTAP * 82 + P - 1) // P          # 2214 -> 18 K-blocks of 128

    L2 = _solve_layer(km1, [c["b_h1"] for c in cores],
                      [c["b_out0"] for c in cores], out0_w_pad)
    L3 = _solve_layer(kmd1, [c["b_out1"] for c in cores],
                      [c["b_h1"] for c in cores], L2["out_w_pad"])
    L4 = _solve_layer(km2, [c["b_h2"] for c in cores],
                      [c["b_out1"] for c in cores], L3["out_w_pad"])
    L5 = _solve_layer(kmd2, [c["b_out2"] for c in cores],
                      [c["b_h2"] for c in cores], L4["out_w_pad"])
    Ls = [L2, L3, L4, L5]

    # fused weights: W_big[(k,(j,d) | hit), co] = (W0[j] @ Wd0[k]) / b0@Wd0[k]
    W0 = np.asarray(inputs["W0"], np.float32)           # [27,3,64]
    Wd0 = np.asarray(inputs["Wd0"], np.float32)         # [27,64,128]
    b0 = np.asarray(inputs["b0"], np.float32)
    wbig = np.zeros((KB * P, P), np.float32)
    for k in range(NTAP):
        pw = np.einsum("jdc,cn->jdn", W0, Wd0[k]).reshape(NTAP * 3, P)
        wbig[k * 82:k * 82 + 81, :] = pw
        wbig[k * 82 + 81, :] = b0 @ Wd0[k]
    # tap weights for the 4 gather layers
    wts = np.zeros((KB + 4 * NTAP, P, P), np.float32)
    wts[:KB] = wbig.reshape(KB, P, P)
    for i, (wn, cin) in enumerate([("W1", 128), ("Wd1", 128),
                                   ("W2", 128), ("Wd2", 128)]):
        w = np.asarray(inputs[wn], np.float32)
        wts[KB + i * NTAP:KB + (i + 1) * NTAP, :cin, :w.shape[2]] = w
    wts = wts.astype(bf16)
    bias = np.zeros((6, P), np.float32)
    for i, bn in enumerate(["b0", "bd0", "b1", "bd1", "b2", "bd2"]):
        b = np.asarray(inputs[bn], np.float32)
        bias[i, :b.shape[0]] = b

    in_maps = []
    for c in range(ncores):
        cc = cores[c]
        # host-composed G1T [KB*128, out0_w_pad]: per out0 row n the x-values
        # of its 27x27 two-hop neighborhood plus per-tap hit bits
        olo, ohi = cc["b_out0"]
        w = ohi - olo
        g1t = np.zeros((KB * P, out0_w_pad), bf16)
        for k in range(NTAP):
            mid = kmd0[k, olo:ohi].astype(np.int64)
            mk = mid >= 0
            r2 = km0[:, np.clip(mid, 0, None)]          # [27, w]
            r2 = np.where(mk[None, :], r2, -1)
            vals = x[np.clip(r2, 0, None)]              # [27, w, 3]
            vals = np.where(r2[:, :, None] >= 0, vals, 0.0)
            g1t[k * 82:k * 82 + 81, :w] = (
                vals.transpose(0, 2, 1).reshape(NTAP * 3, w).astype(bf16))
            g1t[k * 82 + 81, :w] = mk.astype(bf16)
        m = dict(
            g1t=np.ascontiguousarray(g1t),
            wts=wts, bias=bias,
            idx1=_layer_idx(km1, cc["b_h1"], cc["b_out0"], L2),
            idx2=_layer_idx(kmd1, cc["b_out1"], cc["b_h1"], L3),
            idx3=_layer_idx(km2, cc["b_h2"], cc["b_out1"], L4),
            idx4=_layer_idx(kmd2, cc["b_out2"], cc["b_h2"], L5),
        )
        in_maps.append(m)

    meta = dict(
        nchunkf=nchunkf, kb=KB,
        layers=[dict(nchunk=L["nchunk"], bstep=L["bstep"],
                     prepad=L["prepad"], stored_n=L["stored_n"]) for L in Ls],
    )
    asm = dict(cores=cores, Ls=Ls,
               n_out0=kmd0.shape[1], n_out1=kmd1.shape[1],
               n_out2=kmd2.shape[1])
    return in_maps, meta, asm


# ------------------------------------------------------------- bass program
def build_bass(meta):
    key = repr(meta)
    if key in _BUILD_CACHE:
        return _BUILD_CACHE[key]
    import concourse.bass as bass
    import concourse.bacc as bacc
    import concourse.mybir as mybir
    import concourse.tile as tile
    from concourse.bass import ds

    bf = mybir.dt.bfloat16
    f32 = mybir.dt.float32
    nchunkf = meta["nchunkf"]
    KB = meta["kb"]
    Ls = meta["layers"]

    nc = bacc.Bacc("TRN2", target_bir_lowering=False, debug=False)
    g1t_d = nc.dram_tensor("g1t", [KB * P, nchunkf * CH], bf,
                           kind="ExternalInput")
    wts_d = nc.dram_tensor("wts", [KB + 4 * NTAP, P, P], bf,
                           kind="ExternalInput")
    bias_d = nc.dram_tensor("bias", [6, P], f32, kind="ExternalInput")
    idx_d = [nc.dram_tensor(f"idx{l + 1}", [Ls[l]["nchunk"] * P,
                                            NTAP * (CH // 16)],
                            mybir.dt.int16, kind="ExternalInput")
             for l in range(4)]
    o0f = nc.dram_tensor("o0f", [nchunkf * CH, P], f32,
                         kind="ExternalOutput")
    o1f = nc.dram_tensor("o1f", [Ls[1]["nchunk"] * CH, P], f32,
                         kind="ExternalOutput")
    o2f = nc.dram_tensor("o2f", [Ls[3]["nchunk"] * CH, P], f32,
                         kind="ExternalOutput")

    with tile.TileContext(nc) as tc:
        with (
            tc.tile_pool(name="cst", bufs=1) as cst,
            tc.tile_pool(name="sb", bufs=3) as sb,
            tc.tile_pool(name="gtp", bufs=8) as gtp,
            tc.tile_pool(name="g1p", bufs=3) as g1p,
            tc.tile_pool(name="ps", bufs=2, space="PSUM") as ps,
            tc.tile_pool(name="dram", bufs=1, space="DRAM") as dram,
        ):
            tbl = [dram.tile([Ls[l]["stored_n"], P], bf, name=f"tbl{l}")
                   for l in range(4)]
            # load constants
            wt = cst.tile([P, KB + 4 * NTAP, P], bf)
            nc.sync.dma_start(out=wt[:], in_=wts_d.rearrange("t k n -> k t n"))
            ones = cst.tile([1, P], bf)
            nc.any.memset(ones[:], 1.0)
            btiles = []
            for i in range(6):
                bt = cst.tile([1, P], bf, tag=f"b{i}")
                nc.gpsimd.dma_start(out=bt[:], in_=bias_d[i:i + 1, :])
                btiles.append(bt)
            zs = cst.tile([1, P], bf)
            nc.any.memset(zs[:], 0.0)
            g1t_r = g1t_d.rearrange("(b p) n -> p b n", p=P)
            # zero rows of each table (segment heads)
            for l in range(4):
                pp, sn = Ls[l]["prepad"], Ls[l]["stored_n"]
                nseg = (sn - pp - 1) // (SEG + 1) + 1
                for gseg in range(nseg):
                    r = pp + (SEG + 1) * gseg
                    nc.sync.dma_start(out=tbl[l][r:r + 1, :], in_=zs[:1, :])

            def epilogue(l, o_ps_list, c, relu, table, fout, prepad):
                """store 4x128-row blocks of chunk c."""
                spb = SEG // P
                for j in range(4):
                    m = c * 4 + j
                    if fout is not None:
                        of = sb.tile([P, P], f32, tag="of")
                        if relu:
                            nc.vector.tensor_relu(out=of[:], in_=o_ps_list[j][:])
                        else:
                            nc.vector.tensor_copy(out=of[:], in_=o_ps_list[j][:])
                        nc.sync.dma_start(out=fout[ds(m * P, P), :], in_=of[:])
                        if table is not None:
                            ob = sb.tile([P, P], bf, tag="ob")
                            nc.vector.tensor_copy(out=ob[:], in_=of[:])
                            nc.sync.dma_start(
                                out=table[ds(m * P + m // spb + prepad + 1, P), :],
                                in_=ob[:])
                    else:
                        ob = sb.tile([P, P], bf, tag="ob")
                        if l == 0:
                            nc.any.memset(ob[:, 64:], 0.0)
                            nc.vector.tensor_copy(out=ob[:, :64],
                                                  in_=o_ps_list[j][:, :64])
                        else:
                            nc.vector.tensor_copy(out=ob[:], in_=o_ps_list[j][:])
                        nc.sync.dma_start(
                            out=table[ds(m * P + m // spb + prepad + 1, P), :],
                            in_=ob[:])

            # ------- fused layer (layer0 + kmd0): out0 = relu(G1 @ Wbig + bd0)
            def body0(c):
                g1 = g1p.tile([P, KB, CH], bf, tag="g1")
                nc.sync.dma_start(out=g1[:], in_=g1t_r[:, :, ds(c * CH, CH)])
                opl = []
                for j in range(4):
                    o_ps = ps.tile([P, P], f32, tag=f"o{j}")
                    nc.tensor.matmul(out=o_ps[:], lhsT=ones[:],
                                     rhs=btiles[1][:], start=True, stop=False)
                    for b in range(KB):
                        nc.tensor.matmul(
                            out=o_ps[:],
                            lhsT=g1[:, b, j * P:(j + 1) * P],
                            rhs=wt[:, b, :], start=False, stop=(b == KB - 1))
                    opl.append(o_ps)
                epilogue(1, opl, c, True, tbl[0], o0f, Ls[0]["prepad"])

            nmain = (nchunkf // UNROLL) * UNROLL
            if nmain:
                with tc.For_i(0, nmain, UNROLL) as c0:
                    for u in range(UNROLL):
                        body0(c0 + u)
            for c in range(nmain, nchunkf):
                body0(c)

            # ---------------- gather layers
            def gather_layer(l, tin, tout, fout, relu, idxs):
                cfg = Ls[l]
                wbase = KB + l * NTAP
                bstep = cfg["bstep"]
                bias_row = btiles[l + 2][:]

                def body(c):
                    it = sb.tile([P, NTAP * (CH // 16)], mybir.dt.int16,
                                 tag="idx")
                    nc.sync.dma_start(out=it[:], in_=idxs[ds(c * P, P), :])
                    opl = []
                    for j in range(4):
                        o_ps = ps.tile([P, P], f32, tag=f"o{j}")
                        nc.tensor.matmul(out=o_ps[:], lhsT=ones[:],
                                         rhs=bias_row, start=True, stop=False)
                        opl.append(o_ps)
                    win_ap = (tin[0:WIN, :] if bstep == 0
                              else tin[ds(c * bstep, WIN), :])
                    for k in range(NTAP):
                        gt = gtp.tile([P, 1, CH], bf, tag="gt")
                        nc.gpsimd.dma_gather(
                            out_ap=gt[:],
                            in_ap=win_ap,
                            idxs_ap=it[:, k * (CH // 16):(k + 1) * (CH // 16)],
                            num_idxs=CH,
                            num_idxs_reg=CH,
                            elem_size=P,
                            transpose=True,
                        )
                        for j in range(4):
                            nc.tensor.matmul(
                                out=opl[j][:],
                                lhsT=gt[:, 0, j * P:(j + 1) * P],
                                rhs=wt[:, wbase + k, :],
                                start=False, stop=(k == NTAP - 1))
                    epilogue(l + 2, opl, c, relu, tout, fout,
                             Ls[l + 1]["prepad"] if l + 1 < 4 else 0)

                nch = cfg["nchunk"]
                nmain = (nch // UNROLL) * UNROLL
                if nmain:
                    with tc.For_i(0, nmain, UNROLL) as c0:
                        for u in range(UNROLL):
                            body(c0 + u)
                for c in range(nmain, nch):
                    body(c)

            gather_layer(0, tbl[0], tbl[1], None, False, idx_d[0])  # km1->h1
            gather_layer(1, tbl[1], tbl[2], o1f, True, idx_d[1])    # kmd1
            gather_layer(2, tbl[2], tbl[3], None, False, idx_d[2])  # km2->h2
            gather_layer(3, tbl[3], None, o2f, False, idx_d[3])     # kmd2

    nc.compile()
    _BUILD_CACHE[key] = nc
    return nc


# -------------------------------------------------------------------- entry
def kernel(**inputs):
    _install_neff_cache()
    from concourse.bass_utils import run_bass_kernel_spmd
    in_maps, meta, asm = make_plan(inputs)
    nc = build_bass(meta)
    res = run_bass_kernel_spmd(nc, in_maps, core_ids=list(range(8)))
    return assemble(asm, [r for r in res.results])


def assemble(asm, results):
    cores = asm["cores"]
    out0 = np.zeros((asm["n_out0"], P), np.float32)
    out1 = np.zeros((asm["n_out1"], P), np.float32)
    out2 = np.zeros((asm["n_out2"], P), np.float32)
    for c, cc in enumerate(cores):
        r = results[c]
        lo, hi = cc["o0"]
        blo = cc["b_out0"][0]
        out0[lo:hi] = r["o0f"][lo - blo:hi - blo]
        lo, hi = cc["o1"]
        blo = cc["b_out1"][0]
        out1[lo:hi] = r["o1f"][lo - blo:hi - blo]
        lo, hi = cc["o2"]
        blo = cc["b_out2"][0]
        out2[lo:hi] = r["o2f"][lo - blo:hi - blo]
    return (out2, out1, out0)


# revision 20
# speedup vs baseline: 1.1874x; 1.1874x over previous
"""Trainium2 Bass kernel for the sparse-conv encoder (gnn_message_passing).

Design:
- 8-way SPMD over contiguous row slabs of each layer's output, with halo
  bands (each core redundantly computes the halo rows later layers gather),
  so there are NO collectives: every core runs an independent program on its
  own band of the point cloud.
- Layers 0+1 (conv0 then kmd0) are FUSED on the host: out0 is linear in
  host-known gathers of x, so out0 = relu(G1 @ Wbig + bd0) exactly, where
  G1[n] holds the 27x27 two-hop x-neighborhood values + per-tap hit bits
  (27x82 = 2214 features, shipped transposed in bf16) and
  Wbig[(k,(j,d)),:] = W0[j] @ Wd0[k] (hit rows b0 @ Wd0[k]) is composed in
  fp32. This removes the two worst gather layers entirely (the gather
  primitives cost ~45 ns per gathered row on the Q7 descriptor path,
  measured, so eliminating indices dominates all other optimizations).
- Layers 2-5 are gather-GEMMs: feature tables live in DRAM as bf16
  row-major with a zero row interleaved every 2049 stored rows. Per output
  chunk of 512 rows and per tap, a dma_gather(transpose=True) pulls the 512
  needed rows directly into SBUF **already transposed** ([C=128 part, 512]),
  which feeds nc.tensor.matmul as the stationary lhsT with the tap weight
  [C_in, C_out] streamed as rhs, accumulating the chunk's [512, C_out]
  output in PSUM across the 27 taps (bias seeded via a K=1 ones x bias
  matmul). dma_gather indices are int16, so each chunk reads through a
  32768-row window whose base advances affinely with the chunk index
  (base = chunk * BSTEP); the host shifts tables by PREPAD rows so one
  compile-time BSTEP per layer covers all 8 cores. Missing taps (-1) are
  remapped by the host to an in-window interleaved zero row.
"""
import hashlib
import os
import numpy as np

P = 128
CH = 512            # output rows per chunk
SEG = 2048          # real rows per zero-row segment (stored period SEG+1)
WIN = 32768         # int16 gather window (rows)
NTAP = 27
UNROLL = 4

_BUILD_CACHE = {}


# ---------------------------------------------------------------- NEFF cache
def _install_neff_cache():
    try:
        import concourse.bass_utils as bu
        import concourse.bass2jax as b2j
        if getattr(bu, "_ant_neff_cache_installed", False):
            return
        cache_dir = os.path.expanduser("~/.cache/ant_neff_cache")
        os.makedirs(cache_dir, exist_ok=True)
        orig = bu.compile_bir_kernel

        def cached(bir_json, tmpdir, neff_name="file.neff", **kw):
            if isinstance(bir_json, bytes):
                key = hashlib.sha256(bir_json).hexdigest()
            elif isinstance(bir_json, str):
                key = hashlib.sha256(bir_json.encode()).hexdigest()
            else:
                return orig(bir_json, tmpdir, neff_name=neff_name, **kw)
            path = os.path.join(cache_dir, key + ".neff")
            out_path = os.path.join(tmpdir, neff_name)
            if os.path.exists(path):
                import shutil
                shutil.copyfile(path, out_path)
                return out_path
            res = orig(bir_json, tmpdir, neff_name=neff_name, **kw)
            try:
                import shutil
                shutil.copyfile(res, path)
            except Exception:
                pass
            return res

        bu.compile_bir_kernel = cached
        b2j.compile_bir_kernel = cached
        bu._ant_neff_cache_installed = True
    except Exception:
        pass


# ---------------------------------------------------------------- host plan
def _np32(a):
    a = np.asarray(a)
    if a.dtype == np.int64:
        a = a.astype(np.int32)
    return a


def _hull(kmap, lo, hi):
    sub = kmap[:, lo:hi]
    v = sub[sub >= 0]
    if v.size == 0:
        return 0, 1
    return int(v.min()), int(v.max()) + 1


def _bands(kmaps, ncores=8):
    km0, kmd0, km1, kmd1, km2, kmd2 = kmaps

    def split(n):
        b = [round(c * n / ncores) for c in range(ncores + 1)]
        return [(b[c], b[c + 1]) for c in range(ncores)]

    owned0 = split(kmd0.shape[1])
    owned1 = split(kmd1.shape[1])
    owned2 = split(kmd2.shape[1])
    cores = []
    for c in range(ncores):
        o2 = owned2[c]
        b_h2 = _hull(kmd2, *o2)
        h_km2 = _hull(km2, *b_h2)
        o1 = owned1[c]
        b_out1 = (min(o1[0], h_km2[0]), max(o1[1], h_km2[1]))
        b_h1 = _hull(kmd1, *b_out1)
        h_km1 = _hull(km1, *b_h1)
        o0 = owned0[c]
        b_out0 = (min(o0[0], h_km1[0]), max(o0[1], h_km1[1]))
        b_h0 = _hull(kmd0, *b_out0)
        cores.append(dict(b_h0=b_h0, b_out0=b_out0, o0=o0,
                          b_h1=b_h1, b_out1=b_out1, o1=o1,
                          b_h2=b_h2, b_out2=o2, o2=o2))
    return cores


def _spos(r, prepad):
    """local real row -> stored row (zero row every SEG real rows)."""
    return prepad + 1 + r + r // SEG


def _solve_layer(kmap, out_bands, in_bands, writer_w_pad):
    """Pick BSTEP/PREPAD for one gather layer. Returns dict of constants."""
    ncores = len(out_bands)
    out_w = max(hi - lo for lo, hi in out_bands)
    nchunk = -(-out_w // CH)
    out_w_pad = nchunk * CH
    lo_l, hi_l, cks = [], [], []
    for c in range(ncores):
        olo, ohi = out_bands[c]
        ilo, _ = in_bands[c]
        w = ohi - olo
        sub = kmap[:, olo:ohi]
        loc = np.where(sub >= 0, (sub - ilo) + (sub - ilo) // SEG + 1, -1)
        for ck in range(nchunk):
            s, e = ck * CH, min((ck + 1) * CH, w)
            if s >= w:
                continue
            v = loc[:, s:e]
            v = v[v >= 0]
            if v.size == 0:
                continue
            lo_l.append(int(v.min()))
            hi_l.append(int(v.max()) + 1)
            cks.append(ck)
    lo_a = np.array(lo_l)
    hi_a = np.array(hi_l)
    ck_a = np.array(cks)

    def cost(b):
        m1 = max(0, int((ck_a * b - lo_a).max()))
        m2 = int((hi_a - ck_a * b).max())
        return m1 + m2, m1

    # convex in b -> ternary search over ints
    lo_b, hi_b = 0, 4096
    while hi_b - lo_b > 2:
        m1 = lo_b + (hi_b - lo_b) // 3
        m2 = hi_b - (hi_b - lo_b) // 3
        if cost(m1)[0] <= cost(m2)[0]:
            hi_b = m2
        else:
            lo_b = m1
    best = min(range(lo_b, hi_b + 1), key=lambda b: cost(b)[0])
    tot, prepad = cost(best)
    assert tot <= WIN - 64, f"window infeasible: {tot} > {WIN}"
    bstep = int(best)
    # stored table size: written rows + window reach
    stored_n = max(prepad + 1 + writer_w_pad + writer_w_pad // SEG + 1,
                   (nchunk - 1) * bstep + WIN, WIN)
    stored_n = -(-stored_n // P) * P
    return dict(nchunk=nchunk, out_w_pad=out_w_pad, bstep=bstep,
                prepad=prepad, stored_n=stored_n)


def _wrap_idx(rel):
    """rel [nchunk, NTAP, CH] int -> [nchunk*128, NTAP*CH//16] int16
    (16-partition wrap, replicated x8)."""
    nchunk = rel.shape[0]
    a = rel.reshape(nchunk, NTAP, CH // 16, 16).transpose(0, 3, 1, 2)
    a = a.reshape(nchunk, 1, 16, NTAP * (CH // 16))
    a = np.broadcast_to(a, (nchunk, 8, 16, NTAP * (CH // 16)))
    return np.ascontiguousarray(
        a.reshape(nchunk * P, NTAP * (CH // 16)), ).astype(np.int16)


def _layer_idx(kmap, out_band, in_band, cfg):
    """Build the wrapped int16 index buffer for one core of one layer."""
    olo, ohi = out_band
    ilo, _ = in_band
    w = ohi - olo
    nchunk, bstep, prepad = cfg["nchunk"], cfg["bstep"], cfg["prepad"]
    sub = kmap[:, olo:ohi].astype(np.int64)
    loc = np.where(sub >= 0, prepad + 1 + (sub - ilo) + (sub - ilo) // SEG, -1)
    full = np.full((NTAP, nchunk * CH), -1, np.int64)
    full[:, :w] = loc
    rel = np.empty((nchunk, NTAP, CH), np.int64)
    max_base = cfg["stored_n"] - WIN
    for ck in range(nchunk):
        base = ck * bstep
        assert 0 <= base <= max_base, (ck, base, max_base)
        g = max(0, -(-(base - prepad) // (SEG + 1)))
        zr = prepad + (SEG + 1) * g
        assert base <= zr < base + WIN and zr < cfg["stored_n"]
        v = full[:, ck * CH:(ck + 1) * CH] - base
        v = np.where(full[:, ck * CH:(ck + 1) * CH] < 0, zr - base, v)
        assert v.min() >= 0 and v.max() < WIN, (ck, v.min(), v.max())
        rel[ck] = v
    return _wrap_idx(rel)


def make_plan(inputs, ncores=8):
    import ml_dtypes
    bf16 = ml_dtypes.bfloat16
    x = np.asarray(inputs["x"], np.float32)
    kmaps = [_np32(inputs[k]) for k in
             ["km0", "kmd0", "km1", "kmd1", "km2", "kmd2"]]
    km0, kmd0, km1, kmd1, km2, kmd2 = kmaps
    cores = _bands(kmaps, ncores)

    # fused (layer0 + kmd0) output band: b_out0, padded
    out0_w = max(c["b_out0"][1] - c["b_out0"][0] for c in cores)
    nchunkf = -(-out0_w // CH)
    out0_w_pad = nchunkf * CH
    KB = (NTAP * 82 + P - 1) // P          # 2214 -> 18 K-blocks of 128

    L2 = _solve_layer(km1, [c["b_h1"] for c in cores],
                      [c["b_out0"] for c in cores], out0_w_pad)
    L3 = _solve_layer(kmd1, [c["b_out1"] for c in cores],
                      [c["b_h1"] for c in cores], L2["out_w_pad"])
    L4 = _solve_layer(km2, [c["b_h2"] for c in cores],
                      [c["b_out1"] for c in cores], L3["out_w_pad"])
    L5 = _solve_layer(kmd2, [c["b_out2"] for c in cores],
                      [c["b_h2"] for c in cores], L4["out_w_pad"])
    Ls = [L2, L3, L4, L5]

    # fused weights: W_big[(k,(j,d) | hit), co] = (W0[j] @ Wd0[k]) / b0@Wd0[k]
    W0 = np.asarray(inputs["W0"], np.float32)           # [27,3,64]
    Wd0 = np.asarray(inputs["Wd0"], np.float32)         # [27,64,128]
    b0 = np.asarray(inputs["b0"], np.float32)
    wbig = np.zeros((KB * P, P), np.float32)
    for k in range(NTAP):
        pw = np.einsum("jdc,cn->jdn", W0, Wd0[k]).reshape(NTAP * 3, P)
        wbig[k * 82:k * 82 + 81, :] = pw
        wbig[k * 82 + 81, :] = b0 @ Wd0[k]
    # tap weights for the 4 gather layers
    wts = np.zeros((KB + 4 * NTAP, P, P), np.float32)
    wts[:KB] = wbig.reshape(KB, P, P)
    for i, (wn, cin) in enumerate([("W1", 128), ("Wd1", 128),
                                   ("W2", 128), ("Wd2", 128)]):
        w = np.asarray(inputs[wn], np.float32)
        wts[KB + i * NTAP:KB + (i + 1) * NTAP, :cin, :w.shape[2]] = w
    wts = wts.astype(bf16)
    bias = np.zeros((6, P), np.float32)
    for i, bn in enumerate(["b0", "bd0", "b1", "bd1", "b2", "bd2"]):
        b = np.asarray(inputs[bn], np.float32)
        bias[i, :b.shape[0]] = b

    in_maps = []
    for c in range(ncores):
        cc = cores[c]
        # host-composed G1T [KB*128, out0_w_pad]: per out0 row n the x-values
        # of its 27x27 two-hop neighborhood plus per-tap hit bits
        olo, ohi = cc["b_out0"]
        w = ohi - olo
        g1t = np.zeros((KB * P, out0_w_pad), bf16)
        for k in range(NTAP):
            mid = kmd0[k, olo:ohi].astype(np.int64)
            mk = mid >= 0
            r2 = km0[:, np.clip(mid, 0, None)]          # [27, w]
            r2 = np.where(mk[None, :], r2, -1)
            vals = x[np.clip(r2, 0, None)]              # [27, w, 3]
            vals = np.where(r2[:, :, None] >= 0, vals, 0.0)
            g1t[k * 82:k * 82 + 81, :w] = (
                vals.transpose(0, 2, 1).reshape(NTAP * 3, w).astype(bf16))
            g1t[k * 82 + 81, :w] = mk.astype(bf16)
        m = dict(
            g1t=np.ascontiguousarray(g1t),
            wts=wts, bias=bias,
            idx1=_layer_idx(km1, cc["b_h1"], cc["b_out0"], L2),
            idx2=_layer_idx(kmd1, cc["b_out1"], cc["b_h1"], L3),
            idx3=_layer_idx(km2, cc["b_h2"], cc["b_out1"], L4),
            idx4=_layer_idx(kmd2, cc["b_out2"], cc["b_h2"], L5),
        )
        in_maps.append(m)

    meta = dict(
        nchunkf=nchunkf, kb=KB,
        layers=[dict(nchunk=L["nchunk"], bstep=L["bstep"],
                     prepad=L["prepad"], stored_n=L["stored_n"]) for L in Ls],
    )
    asm = dict(cores=cores, Ls=Ls,
               n_out0=kmd0.shape[1], n_out1=kmd1.shape[1],
               n_out2=kmd2.shape[1])
    return in_maps, meta, asm


# ------------------------------------------------------------- bass program
def build_bass(meta):
    key = repr(meta)
    if key in _BUILD_CACHE:
        return _BUILD_CACHE[key]
    import concourse.bass as bass
    import concourse.bacc as bacc
    import concourse.mybir as mybir
    import concourse.tile as tile
    from concourse.bass import ds

    bf = mybir.dt.bfloat16
    f32 = mybir.dt.float32
    nchunkf = meta["nchunkf"]
    KB = meta["kb"]
    Ls = meta["layers"]

    nc = bacc.Bacc("TRN2", target_bir_lowering=False, debug=False)
    g1t_d = nc.dram_tensor("g1t", [KB * P, nchunkf * CH], bf,
                           kind="ExternalInput")
    wts_d = nc.dram_tensor("wts", [KB + 4 * NTAP, P, P], bf,
                           kind="ExternalInput")
    bias_d = nc.dram_tensor("bias", [6, P], f32, kind="ExternalInput")
    idx_d = [nc.dram_tensor(f"idx{l + 1}", [Ls[l]["nchunk"] * P,
                                            NTAP * (CH // 16)],
                            mybir.dt.int16, kind="ExternalInput")
             for l in range(4)]
    o0f = nc.dram_tensor("o0f", [nchunkf * CH, P], f32,
                         kind="ExternalOutput")
    o1f = nc.dram_tensor("o1f", [Ls[1]["nchunk"] * CH, P], f32,
                         kind="ExternalOutput")
    o2f = nc.dram_tensor("o2f", [Ls[3]["nchunk"] * CH, P], f32,
                         kind="ExternalOutput")

    with tile.TileContext(nc) as tc:
        with (
            tc.tile_pool(name="cst", bufs=1) as cst,
            tc.tile_pool(name="sb", bufs=3) as sb,
            tc.tile_pool(name="gtp", bufs=8) as gtp,
            tc.tile_pool(name="g1p", bufs=3) as g1p,
            tc.tile_pool(name="ps", bufs=2, space="PSUM") as ps,
            tc.tile_pool(name="dram", bufs=1, space="DRAM") as dram,
        ):
            tbl = [dram.tile([Ls[l]["stored_n"], P], bf, name=f"tbl{l}")
                   for l in range(4)]
            # load constants
            wt = cst.tile([P, KB + 4 * NTAP, P], bf)
            nc.sync.dma_start(out=wt[:], in_=wts_d.rearrange("t k n -> k t n"))
            ones = cst.tile([1, P], bf)
            nc.any.memset(ones[:], 1.0)
            btiles = []
            for i in range(6):
                bt = cst.tile([1, P], bf, tag=f"b{i}")
                nc.gpsimd.dma_start(out=bt[:], in_=bias_d[i:i + 1, :])
                btiles.append(bt)
            zs = cst.tile([1, P], bf)
            nc.any.memset(zs[:], 0.0)
            from concourse.masks import make_identity
            ident = cst.tile([P, P], bf)
            make_identity(nc, ident)
            g1t_r = g1t_d.rearrange("(b p) n -> p b n", p=P)
            # zero rows of each table (segment heads)
            for l in range(4):
                pp, sn = Ls[l]["prepad"], Ls[l]["stored_n"]
                nseg = (sn - pp - 1) // (SEG + 1) + 1
                for gseg in range(nseg):
                    r = pp + (SEG + 1) * gseg
                    nc.sync.dma_start(out=tbl[l][r:r + 1, :], in_=zs[:1, :])

            def epilogue(l, o_ps_list, c, relu, table, fout, prepad):
                """store 4x128-row blocks of chunk c."""
                spb = SEG // P
                for j in range(4):
                    m = c * 4 + j
                    if fout is not None:
                        of = sb.tile([P, P], f32, tag="of")
                        if relu:
                            nc.vector.tensor_relu(out=of[:], in_=o_ps_list[j][:])
                        else:
                            nc.vector.tensor_copy(out=of[:], in_=o_ps_list[j][:])
                        nc.sync.dma_start(out=fout[ds(m * P, P), :], in_=of[:])
                        if table is not None:
                            ob = sb.tile([P, P], bf, tag="ob")
                            nc.vector.tensor_copy(out=ob[:], in_=of[:])
                            nc.sync.dma_start(
                                out=table[ds(m * P + m // spb + prepad + 1, P), :],
                                in_=ob[:])
                    else:
                        ob = sb.tile([P, P], bf, tag="ob")
                        if l == 0:
                            nc.any.memset(ob[:, 64:], 0.0)
                            nc.vector.tensor_copy(out=ob[:, :64],
                                                  in_=o_ps_list[j][:, :64])
                        else:
                            nc.vector.tensor_copy(out=ob[:], in_=o_ps_list[j][:])
                        nc.sync.dma_start(
                            out=table[ds(m * P + m // spb + prepad + 1, P), :],
                            in_=ob[:])

            # ------- fused layer (layer0 + kmd0): out0 = relu(G1 @ Wbig + bd0)
            def body0(c):
                g1 = g1p.tile([P, KB, CH], bf, tag="g1")
                nc.sync.dma_start(out=g1[:], in_=g1t_r[:, :, ds(c * CH, CH)])
                opl = []
                for j in range(4):
                    o_ps = ps.tile([P, P], f32, tag=f"o{j}", bufs=1)
                    nc.tensor.matmul(out=o_ps[:], lhsT=ones[:],
                                     rhs=btiles[1][:], start=True, stop=False)
                    for b in range(KB):
                        nc.tensor.matmul(
                            out=o_ps[:],
                            lhsT=g1[:, b, j * P:(j + 1) * P],
                            rhs=wt[:, b, :], start=False, stop=(b == KB - 1))
                    opl.append(o_ps)
                epilogue(1, opl, c, True, tbl[0], o0f, Ls[0]["prepad"])

            nmain = (nchunkf // UNROLL) * UNROLL
            if nmain:
                with tc.For_i(0, nmain, UNROLL) as c0:
                    for u in range(UNROLL):
                        body0(c0 + u)
            for c in range(nmain, nchunkf):
                body0(c)

            # ---------------- gather layers
            def gather_layer(l, tin, tout, fout, relu, idxs):
                cfg = Ls[l]
                wbase = KB + l * NTAP
                bstep = cfg["bstep"]
                bias_row = btiles[l + 2][:]

                def body(c):
                    it = sb.tile([P, NTAP * (CH // 16)], mybir.dt.int16,
                                 tag="idx")
                    nc.sync.dma_start(out=it[:], in_=idxs[ds(c * P, P), :])
                    opl = []
                    for j in range(4):
                        o_ps = ps.tile([P, P], f32, tag=f"o{j}", bufs=1)
                        nc.tensor.matmul(out=o_ps[:], lhsT=ones[:],
                                         rhs=bias_row, start=True, stop=False)
                        opl.append(o_ps)
                    win_ap = (tin[0:WIN, :] if bstep == 0
                              else tin[ds(c * bstep, WIN), :])
                    for k in range(NTAP):
                        gt = gtp.tile([P, 4, P], bf, tag="gt")
                        nc.gpsimd.dma_gather(
                            out_ap=gt[:],
                            in_ap=win_ap,
                            idxs_ap=it[:, k * (CH // 16):(k + 1) * (CH // 16)],
                            num_idxs=CH,
                            num_idxs_reg=CH,
                            elem_size=P,
                            transpose=False,
                        )
                        for j in range(4):
                            tp = ps.tile([P, P], bf, tag="tp", bufs=4)
                            nc.tensor.transpose(out=tp[:], in_=gt[:, j, :],
                                                identity=ident[:])
                            tsb = sb.tile([P, P], bf, tag="tsb")
                            nc.vector.tensor_copy(out=tsb[:], in_=tp[:])
                            nc.tensor.matmul(
                                out=opl[j][:],
                                lhsT=tsb[:],
                                rhs=wt[:, wbase + k, :],
                                start=False, stop=(k == NTAP - 1))
                    epilogue(l + 2, opl, c, relu, tout, fout,
                             Ls[l + 1]["prepad"] if l + 1 < 4 else 0)

                nch = cfg["nchunk"]
                nmain = (nch // UNROLL) * UNROLL
                if nmain:
                    with tc.For_i(0, nmain, UNROLL) as c0:
                        for u in range(UNROLL):
                            body(c0 + u)
                for c in range(nmain, nch):
                    body(c)

            gather_layer(0, tbl[0], tbl[1], None, False, idx_d[0])  # km1->h1
            gather_layer(1, tbl[1], tbl[2], o1f, True, idx_d[1])    # kmd1
            gather_layer(2, tbl[2], tbl[3], None, False, idx_d[2])  # km2->h2
            gather_layer(3, tbl[3], None, o2f, False, idx_d[3])     # kmd2

    nc.compile()
    _BUILD_CACHE[key] = nc
    return nc


# -------------------------------------------------------------------- entry
def kernel(**inputs):
    _install_neff_cache()
    from concourse.bass_utils import run_bass_kernel_spmd
    in_maps, meta, asm = make_plan(inputs)
    nc = build_bass(meta)
    res = run_bass_kernel_spmd(nc, in_maps, core_ids=list(range(8)))
    return assemble(asm, [r for r in res.results])


def assemble(asm, results):
    cores = asm["cores"]
    out0 = np.zeros((asm["n_out0"], P), np.float32)
    out1 = np.zeros((asm["n_out1"], P), np.float32)
    out2 = np.zeros((asm["n_out2"], P), np.float32)
    for c, cc in enumerate(cores):
        r = results[c]
        lo, hi = cc["o0"]
        blo = cc["b_out0"][0]
        out0[lo:hi] = r["o0f"][lo - blo:hi - blo]
        lo, hi = cc["o1"]
        blo = cc["b_out1"][0]
        out1[lo:hi] = r["o1f"][lo - blo:hi - blo]
        lo, hi = cc["o2"]
        blo = cc["b_out2"][0]
        out2[lo:hi] = r["o2f"][lo - blo:hi - blo]
    return (out2, out1, out0)
